# revision 40
# baseline (speedup 1.0000x reference)
"""NetMamba feature extractor on 8 TRN2 NeuronCores (pure data parallelism).

Self-contained: hardcodes all shapes. kernel(**inputs) takes the FULL inputs
(as produced by the problem's setup_inputs) and returns the FULL output
[256, 256] (cls-token features after the final rmsnorm).

Sharding: batch 256 -> 32 per core, params replicated, no collectives.

Per-core layout: channel-major [d (2 blocks of 128 partitions), token
(b-major, l-minor, L=51 incl. cls at l=0)].  The SSM scan runs as one DVE
tensor_tensor_scan per (d-block, b-chunk) over the packed (b, n, l) free
dim, with dA zeroed at l=0 of each (b, n) group so the independent
recurrences chain correctly through a single instruction.
"""
import sys
sys.path.insert(0, '/opt/trn_rl_repo')

import numpy as np

import concourse.bass as bass
import concourse.tile as tile
from concourse.tile import add_dep_helper
from concourse import mybir
from concourse.masks import make_identity
from concourse.bass_utils import run_bass_kernel_spmd
import bass_rust as _bass_rust


def _split_waits(nc):
    """Walrus encodes ~one semaphore wait per TPB instruction (large wait
    values cannot spill). Move extra waits onto injected same-engine NoOps
    placed immediately before the instruction — same-engine streams are
    in-order, so semantics are unchanged."""
    skip = ('InstEventSemaphore', 'InstCall',
            'InstUnconditionalBranch', 'InstRegisterMove',
            'InstISA', 'InstPartitionBroadcast')
    k = 0
    for f in nc.m.functions:
        for bb in f.blocks:
            new = []
            for inst in bb.instructions:
                si = inst.sync_info
                import os as _os
                _eng_ok = _os.environ.get('WSPLIT_ENGINES', 'ALL')
                _ename = str(getattr(inst, 'engine', ''))
                if (si is not None and len(si.on_wait) > 1
                        and type(inst).__name__ not in skip
                        and (_eng_ok == 'ALL' or any(e in _ename for e in _eng_ok.split(',')))):
                    waits = list(si.on_wait)
                    if str(inst.engine) in ('EngineType.SP', 'EngineType.Pool'):
                        # SP/Pool NoOps mis-lower; use Drains (one wait
                        # each: only an instruction's first wait gets the
                        # full 32-bit immediate).
                        for w in waits[:-1]:
                            dr = _bass_rust.InstDrain(
                                name=f"wsplit-{k}", ins=[], outs=[])
                            k += 1
                            dr.engine = inst.engine
                            dr.is_reset_sema = False
                            dr.debug = inst.debug
                            dr.sync_info = mybir.SyncInfo(on_wait=[w],
                                                          on_update=[])
                            new.append(dr)
                    else:
                        for w in waits[:-1]:
                            nop = _bass_rust.InstNoOp(
                                name=f"wsplit-{k}", ins=[], outs=[])
                            k += 1
                            nop.engine = inst.engine
                            nop.bass_nofuse = True
                            nop.debug = inst.debug
                            nop.sync_info = mybir.SyncInfo(on_wait=[w],
                                                           on_update=[])
                            new.append(nop)
                    inst.sync_info = mybir.SyncInfo(
                        on_wait=[waits[-1]], on_update=list(si.on_update))
                new.append(inst)
            bb.instructions = new
    return k

F32 = mybir.dt.float32
BF16 = mybir.dt.bfloat16
AF = mybir.ActivationFunctionType
OP = mybir.AluOpType

B, SEQ, D, DEPTH = 256, 50, 256, 4
N, K, R = 16, 4, 16
EPS = 1e-5
L = SEQ + 1                 # 51 tokens
NCORES = 8
BL = B // NCORES            # 32 per-core batch
T = BL * L                  # 1632 tokens per core
BC = 4                      # b-chunk for the scan
NCH = BL // BC              # 8 scan chunks
CT = BC * L                 # 204 tokens per chunk
SC = 4                      # b-superchunk for dA writes
NSC = BL // SC              # 4 superchunks
NB = 4                      # token chunks for matmuls
TB = T // NB                # 408


def _build(tc, nc, io):
    o_out = io['o_out']
    ctxs = []

    def pool(name, bufs, space="SBUF"):
        p = tc.tile_pool(name=name, bufs=bufs, space=space)
        ctxs.append(p)
        return p.__enter__()

    wpool = pool("weights", 1)           # persistent weights/constants
    wrot = pool("wrot", 1)               # rotating big weights (in/out proj)
    apool = pool("acts", 1)              # persistent activations
    spool = pool("scratch", 2)           # 13KB-class rotating scratch
    dapool = pool("dabuf", 1)            # dA superchunk buffer (52KB, + xpad)
    hspool = pool("hsbuf", 1)            # scan output / xdbl_dt
    reppool = pool("rep", 1)             # replicated Bm/C per chunk
    smpool = pool("small", 2)            # small rotating tiles
    pmm = pool("psmm", 2, "PSUM")        # [128, <=408] matmul outs
    paux = pool("psaux", 1, "PSUM")      # norm/aux psum tiles
    drpool = pool("dram", 2, "DRAM")

    # ---------------- weights / constants ----------------
    w_xp, w_dt, w_cv, a_sc = [], [], [], []
    for i in range(DEPTH):
        t_xp = wpool.tile([128, 2, R + 2 * N], F32, name=f"wxp{i}")
        for kb in range(2):
            nc.sync.dma_start(t_xp[:, kb], io['i_wxp'][i, kb * 128:(kb + 1) * 128])
        w_xp.append(t_xp)
        t_dt = wpool.tile([R, D], F32, name=f"wdt{i}")
        nc.sync.dma_start(t_dt, io['i_wdt'][i])
        w_dt.append(t_dt)
        t_cv = wpool.tile([128, 2, K], F32, name=f"wcv{i}")
        for kb in range(2):
            nc.sync.dma_start(t_cv[:, kb], io['i_cw'][i, kb * 128:(kb + 1) * 128])
        w_cv.append(t_cv)
        t_al = wpool.tile([128, 2, N], F32, name=f"wal{i}")
        for kb in range(2):
            nc.sync.dma_start(t_al[:, kb], io['i_alog'][i, kb * 128:(kb + 1) * 128])
        t_as = wpool.tile([128, 2, N], F32, name=f"was{i}")
        for kb in range(2):
            nc.scalar.activation(t_as[:, kb], t_al[:, kb], AF.Exp)
        nc.vector.tensor_scalar_mul(t_as, t_as, -1.0)
        a_sc.append(t_as)

    nw_t = wpool.tile([128, DEPTH, 2], F32)
    for li in range(DEPTH):
        nc.sync.dma_start(nw_t[:, li], io['i_nw'][li].rearrange("(k p) -> p k", p=128))
    cb_t = wpool.tile([128, DEPTH, 2], F32)
    for li in range(DEPTH):
        nc.sync.dma_start(cb_t[:, li], io['i_cb'][li].rearrange("(k p) -> p k", p=128))
    dtb_t = wpool.tile([128, DEPTH, 2], F32)
    for li in range(DEPTH):
        nc.sync.dma_start(dtb_t[:, li], io['i_dtb'][li].rearrange("(k p) -> p k", p=128))
    dsk_t = wpool.tile([128, DEPTH, 2], F32)
    for li in range(DEPTH):
        nc.sync.dma_start(dsk_t[:, li], io['i_dsk'][li].rearrange("(k p) -> p k", p=128))
    nfw_t = wpool.tile([128, 2], F32)
    nc.sync.dma_start(nfw_t, io['i_nfw'].rearrange("(k p) -> p k", p=128))
    pb_t = wpool.tile([128, 2], F32)
    nc.sync.dma_start(pb_t, io['i_pb'].rearrange("(k p) -> p k", p=128))
    cls_t = wpool.tile([128, 2], F32)
    nc.sync.dma_start(cls_t, io['i_cls'][0, 0].rearrange("(k p) -> p k", p=128))
    pos_t = wpool.tile([128, 2, L], F32)
    for kb in range(2):
        nc.sync.dma_start(
            pos_t[:, kb],
            io['i_pos'][0, :, kb * 128:(kb + 1) * 128].transpose([1, 0]))
    pw_t = wpool.tile([1, D], F32)
    nc.sync.dma_start(pw_t, io['i_pw'])

    ident = wpool.tile([128, 128], F32)
    make_identity(nc, ident)
    ones_col = wpool.tile([128, 1], F32)
    nc.vector.memset(ones_col, 1.0)
    ones_row = wpool.tile([1, 128], F32)
    nc.vector.memset(ones_row, 1.0)
    ones_row_bf = wpool.tile([1, 128], BF16)
    nc.vector.memset(ones_row_bf, 1.0)
    eps_t = wpool.tile([128, 1], F32)
    nc.vector.memset(eps_t, EPS)

    # Touch DMA-loaded tiles on DVE so its observed queue ticks cover the
    # preload: DVE instructions have a single hardware wait slot, so every
    # later op must carry at most one fresh semaphore dependency.
    dummy = wpool.tile([128, 16], F32)
    _tc_i = [0]
    def touch(ap):
        j = _tc_i[0] % 16
        _tc_i[0] += 1
        nc.vector.tensor_copy(out=dummy[0:ap.shape[0], j:j + 1], in_=ap)
    for tch in (nw_t[:, 0, 0:1], cb_t[:, 0, 0:1], dtb_t[:, 0, 0:1],
                dsk_t[:, 0, 0:1], nfw_t[:, 0:1], pb_t[:, 0:1],
                cls_t[:, 0:1], pos_t[:, 0, 0:1], pw_t[:, 0:1]):
        touch(tch)
    for i in range(DEPTH):
        touch(w_cv[i][:, 0, 0:1])

    cls_plus = wpool.tile([128, 2], F32)
    for kb in range(2):
        nc.vector.tensor_tensor(out=cls_plus[:, kb:kb + 1],
                                in0=cls_t[:, kb:kb + 1],
                                in1=pos_t[:, kb, SEQ:SEQ + 1], op=OP.add)

    # ---------------- persistent activations ----------------
    res = apool.tile([128, 2, T], F32)       # residual stream
    z_t = apool.tile([128, 2, T], F32)       # silu(z) -> gated value
    xc_t = apool.tile([128, 2, T], F32)      # conv output
    dt_t = apool.tile([128, 2, T], F32)      # softplus dt
    res4 = res.rearrange("p k (b l) -> p k b l", b=BL)

    # ---------------- embedding ----------------
    x_t = smpool.tile([1, BL * SEQ], F32, tag="row", bufs=1)
    nc.sync.dma_start(x_t, io['i_x'].rearrange("b s -> (b s)").unsqueeze(0))
    wrm0 = paux.tile([1, 128], F32, tag="rowps", bufs=1)
    nc.tensor.matmul(wrm0[0:1, 0:1], ident[0:1, 0:1], ident[0:1, 0:1],
                     start=True, stop=True)
    wrm = paux.tile([1, 128], F32, tag="rowps", bufs=1)
    nc.tensor.matmul(wrm[0:1, 0:1], pw_t[0:1, 0:1], pw_t[0:1, 0:1],
                     start=True, stop=True)
    for kb in range(2):
        for nb in range(4):
            ps = pmm.tile([128, 400], F32, tag="mm")
            nc.tensor.matmul(ps, pw_t[:, kb * 128:(kb + 1) * 128],
                             x_t[:, nb * 400:(nb + 1) * 400],
                             start=True, stop=True)
            nc.vector.tensor_scalar_add(
                out=res4[:, kb, nb * 8:(nb + 1) * 8, 1:],
                in0=ps.rearrange("p (b l) -> p b l", b=8),
                scalar1=pb_t[:, kb:kb + 1])
        nc.vector.tensor_tensor(
            out=res4[:, kb, :, 1:], in0=res4[:, kb, :, 1:],
            in1=pos_t[:, kb, 0:SEQ].unsqueeze(1).broadcast_to([128, BL, SEQ]),
            op=OP.add)
        nc.vector.memset(res4[:, kb, :, 0:1], 0.0)
        nc.vector.tensor_scalar_add(out=res4[:, kb, :, 0],
                                    in0=res4[:, kb, :, 0],
                                    scalar1=cls_plus[:, kb:kb + 1])

    import os as _os
    DBG = _os.environ.get('DEBUG_OUTS')
    if DBG:
        nc.sync.dma_start(io['dbg_emb'], res)

    # ---------------- rmsnorm inverse factor ----------------
    def rms_inv_row(sq_fn, ntok, tag):
        """SBUF [1, ntok] row of 1/sqrt(mean_d(sq)+eps); sq_fn(kb)->[128,ntok]."""
        nblk = (ntok + 127) // 128
        msT = pmm.tile([128, nblk], F32, tag="mm")
        nc.vector.memset(msT, 1.0)
        for j in range(nblk):
            mc = min(128, ntok - j * 128)
            for kb in range(2):
                nc.tensor.matmul(msT[0:mc, j:j + 1],
                                 sq_fn(kb)[:, j * 128:j * 128 + mc], ones_col,
                                 start=(kb == 0), stop=(kb == 1))
        inv = smpool.tile([128, 13], F32, tag="inv")
        # 1/sqrt(m) = exp(-0.5*ln(m)); ln & exp live in one ACT LUT set
        nc.scalar.activation(inv[:, 0:nblk], msT, AF.Ln,
                             bias=eps_t, scale=1.0 / D)
        nc.scalar.activation(inv[:, 0:nblk], inv[:, 0:nblk], AF.Exp,
                             scale=-0.5)
        row = smpool.tile([1, ntok], F32, tag="row", bufs=1)
        for j in range(nblk):
            mc = min(128, ntok - j * 128)
            rps = paux.tile([1, 128], F32, tag="rowps", bufs=1)
            nc.tensor.transpose(rps, inv[:, j:j + 1], ident)
            nc.scalar.copy(row[:, j * 128:j * 128 + mc], rps[:, 0:mc])
        return row

    # ---------------- layers ----------------
    h4 = None
    for i in range(DEPTH):
        last = (i == DEPTH - 1)
        w_in = wrot.tile([128, 2, 2 * D], F32, tag="win")
        w_out = wrot.tile([128, 2, D], F32, tag="wout")
        for kb in range(2):
            nc.sync.dma_start(w_in[:, kb], io['i_win'][i, kb * 128:(kb + 1) * 128])
            nc.sync.dma_start(w_out[:, kb], io['i_wout'][i, kb * 128:(kb + 1) * 128])
        wrm = paux.tile([1, 128], F32, tag="rowps", bufs=1)
        nc.tensor.matmul(wrm[0:1, 0:1], w_in[0:1, 0, 0:1], w_in[0:1, 0, 0:1],
                         start=True, stop=True)
        wrm2 = paux.tile([1, 128], F32, tag="rowps", bufs=1)
        nc.tensor.matmul(wrm2[0:1, 0:1], w_out[0:1, 0, 0:1], w_out[0:1, 0, 0:1],
                         start=True, stop=True)

        # ---- rmsnorm(res) -> hn ----
        sq = spool.tile([128, 2, T], F32, tag="big")
        for kb in range(2):
            nc.scalar.activation(sq[:, kb], res[:, kb], AF.Square)
        inv_row = rms_inv_row(lambda kb: sq[:, kb], T, tag="row")
        hn = spool.tile([128, 2, T], F32, tag="big")
        for nb in range(NB):
            sl = slice(nb * TB, (nb + 1) * TB)
            irep = pmm.tile([128, TB], F32, tag="mm")
            nc.tensor.matmul(irep, ones_row, inv_row[:, sl],
                             start=True, stop=True)
            for kb in range(2):
                nc.vector.scalar_tensor_tensor(
                    out=hn[:, kb, sl], in0=res[:, kb, sl],
                    scalar=nw_t[:, i, kb:kb + 1], in1=irep,
                    op0=OP.mult, op1=OP.mult)

        if DBG and i == 0:
            nc.sync.dma_start(io['dbg_hn'], hn)
        # ---- in_proj ----
        xpad = dapool.tile([128, 2, BL, K - 1 + L], F32, tag="dabuf")
        nc.vector.memset(xpad[:, :, :, 0:K - 1], 0.0)
        for mb in range(4):
            for nb in range(NB):
                ps = pmm.tile([128, TB], F32, tag="mm")
                for kb in range(2):
                    nc.tensor.matmul(
                        ps, w_in[:, kb, mb * 128:(mb + 1) * 128],
                        hn[:, kb, nb * TB:(nb + 1) * TB],
                        start=(kb == 0), stop=(kb == 1))
                if mb < 2:   # x half -> xpad, shifted by K-1
                    nc.scalar.copy(xpad[:, mb, nb * 8:(nb + 1) * 8, K - 1:],
                                   ps.rearrange("p (b l) -> p b l", b=8))
                else:        # z half -> silu on eviction
                    nc.scalar.activation(
                        z_t[:, mb - 2, nb * TB:(nb + 1) * TB], ps, AF.Silu)

        # ---- depthwise causal conv + silu ----
        cacc = spool.tile([128, 2, T], F32, tag="big")
        cacc4 = cacc.rearrange("p k (b l) -> p k b l", b=BL)
        for kb in range(2):
            nc.vector.tensor_scalar_mul(
                out=cacc4[:, kb], in0=xpad[:, kb, :, 0:L],
                scalar1=w_cv[i][:, kb, 0:1])
            for k in range(1, K):
                nc.vector.scalar_tensor_tensor(
                    out=cacc4[:, kb], in0=xpad[:, kb, :, k:k + L],
                    scalar=w_cv[i][:, kb, k:k + 1], in1=cacc4[:, kb],
                    op0=OP.mult, op1=OP.add)
            nc.scalar.activation(xc_t[:, kb], cacc[:, kb], AF.Silu,
                                 bias=cb_t[:, i, kb:kb + 1])

        if DBG and i == 0:
            nc.sync.dma_start(io['dbg_xc'], xc_t)
        # ---- x_proj: dt rows (M=16) and merged B/C rows (M=32, bf16) ----
        xdbl_dt = hspool.tile([R, T], F32, tag="hs", bufs=2)
        xdbl_bc = apool.tile([32, T], BF16, tag="xdblbc")
        for part in range(2):
            for nb in range(NB):
                sl = slice(nb * TB, (nb + 1) * TB)
                ps = paux.tile([32, TB], F32, tag="mm16", bufs=1)
                m0 = R if part else 0
                mw = 2 * N if part else R
                for kb in range(2):
                    nc.tensor.matmul(
                        ps[0:mw], w_xp[i][:, kb, m0:m0 + mw],
                        xc_t[:, kb, sl], start=(kb == 0), stop=(kb == 1))
                if part == 0:
                    nc.scalar.copy(xdbl_dt[:, sl], ps[0:R])
                else:
                    nc.scalar.copy(xdbl_bc[:, sl], ps)
        bc_dram = drpool.tile([32, T], BF16, tag="bcd")
        nc.sync.dma_start(bc_dram, xdbl_bc)

        # ---- dt = softplus(...) ----
        for mb in range(2):
            for nb in range(NB):
                sl = slice(nb * TB, (nb + 1) * TB)
                ps = pmm.tile([128, TB], F32, tag="mm")
                nc.tensor.matmul(ps, w_dt[i][:, mb * 128:(mb + 1) * 128],
                                 xdbl_dt[:, sl], start=True, stop=True)
                # softplus(x) = ln(1 + exp(x))
                spt = smpool.tile([128, TB], F32, tag="spt", bufs=1)
                nc.scalar.activation(spt, ps, AF.Exp,
                                     bias=dtb_t[:, i, mb:mb + 1])
                nc.scalar.activation(dt_t[:, mb, sl], spt, AF.Ln, bias=1.0)

        if DBG and i == 0:
            nc.sync.dma_start(io['dbg_dt'], dt_t)
            nc.sync.dma_start(io['dbg_zs'], z_t)
        # ---- SSM scan ----
        dtx_t = apool.tile([128, 2, T], F32, tag="dtxt")
        for kb in range(2):
            nc.vector.tensor_tensor(out=dtx_t[:, kb], in0=dt_t[:, kb],
                                    in1=xc_t[:, kb], op=OP.mult)
        dt4 = dt_t.rearrange("p k (b l) -> p k b l", b=BL)
        for sc in range(NSC):
            dA = dapool.tile([128, 2, SC, N, L], F32, tag="dabuf")
            for kb in range(2):
                for n in range(N):
                    nc.scalar.activation(
                        dA[:, kb, :, n, :],
                        dt4[:, kb, sc * SC:(sc + 1) * SC, :], AF.Exp,
                        scale=a_sc[i][:, kb, n:n + 1])
            nc.vector.memset(dA[:, :, :, :, 0:1], 0.0)
            for half in range(SC // BC):
                c = sc * (SC // BC) + half
                tsl = slice(c * CT, (c + 1) * CT)
                brow = reppool.tile([1, 2, BC * N * L], BF16, tag="brow")
                nc.sync.dma_start(
                    brow.rearrange("z x (n m) -> z x n m", n=N),
                    bc_dram.rearrange("(x n) t -> x n t", x=2)[:, :, tsl]
                        .unsqueeze(0))

                def rep_psum(bx, hb):
                    # replicate brow row across partitions: K=1 bf16 matmuls.
                    # bank-aligned slots: rp[:, q, 0:408] holds (b2, n4, l)
                    rp = pmm.tile([128, 4, 512], F32, tag="rep", bufs=1)
                    src = brow[0:1, bx].rearrange(
                        "z (n b l) -> z n b l", n=N, b=BC)[:, :, hb * 2:(hb + 1) * 2]
                    srcT = src.transpose([0, 2, 1, 3])   # [z, b2, n, l]
                    for q in range(4):
                        nc.tensor.matmul(rp[:, q, 0:408],
                                         ones_row_bf,
                                         srcT[:, :, q * 4:(q + 1) * 4],
                                         start=True, stop=True)
                    return rp

                dBxs = []
                for kb in range(2):
                    dBxs.append(spool.tile([128, BC, N, L], F32, tag="big",
                                           name=f"dBx{kb}"))
                dtxcs = [dtx_t[:, 0, tsl], dtx_t[:, 1, tsl]]
                for hb in range(2):
                    rp = rep_psum(0, hb)
                    for kb in range(2):
                        hbs = slice(hb * 2, (hb + 1) * 2)
                        for q in range(4):
                            qs = slice(q * 4, (q + 1) * 4)
                            nc.vector.tensor_tensor(
                                out=dBxs[kb][:, hbs, qs],
                                in0=dtxcs[kb]
                                    .rearrange("p (b l) -> p b l", b=BC)
                                    [:, hbs].unsqueeze(2)
                                    .broadcast_to([128, 2, 4, L]),
                                in1=rp[:, q, 0:408].rearrange(
                                    "p (b n l) -> p b n l", b=2, n=4),
                                op=OP.mult)
                hss = []
                for kb in range(2):
                    hs = hspool.tile([128, BC * N * L], F32, tag="hs",
                                     name=f"hs{kb}", bufs=2)
                    nc.vector.tensor_tensor_scan(
                        hs, dA[:, kb, half * BC:(half + 1) * BC]
                            .rearrange("p b n l -> p (b n l)"),
                        dBxs[kb].rearrange("p b n l -> p (b n l)"), 0.0,
                        op0=OP.mult, op1=OP.add)
                    hss.append(hs)
                for hb in range(2):
                    rp = rep_psum(1, hb)
                    for kb in range(2):
                        hbs = slice(hb * 2, (hb + 1) * 2)
                        nc.vector.tensor_tensor(
                            out=dBxs[kb][:, hbs]
                                .rearrange("p b n l -> p b (n l)")
                                .rearrange("p b (q m) -> p b q m", q=4),
                            in0=hss[kb]
                                .rearrange("p (b m) -> p b m", b=BC)[:, hbs]
                                .rearrange("p b (q m) -> p b q m", q=4),
                            in1=rp[:, :, 0:408]
                                .rearrange("p q (b m) -> p b q m", b=2),
                            op=OP.mult)
                for kb in range(2):
                    yc = smpool.tile([128, BC, L], F32, tag="yc")
                    nc.vector.tensor_reduce(
                        out=yc, in_=dBxs[kb].transpose([0, 1, 3, 2]),
                        axis=mybir.AxisListType.X, op=OP.add)
                    ycf = yc.rearrange("p b l -> p (b l)")
                    nc.vector.scalar_tensor_tensor(
                        out=ycf, in0=xc_t[:, kb, tsl],
                        scalar=dsk_t[:, i, kb:kb + 1], in1=ycf,
                        op0=OP.mult, op1=OP.add)
                    nc.vector.tensor_tensor(
                        out=z_t[:, kb, tsl], in0=ycf,
                        in1=z_t[:, kb, tsl], op=OP.mult)

        if DBG and i == 0:
            nc.sync.dma_start(io['dbg_g'], z_t)
        # ---- out_proj: accumulate into res (h4 for last layer) ----
        if last:
            h4 = spool.tile([128, 2, T], F32, tag="big")
        for mb in range(2):
            for nb in range(NB):
                sl = slice(nb * TB, (nb + 1) * TB)
                ps = pmm.tile([128, TB], F32, tag="mm")
                for kb in range(2):
                    nc.tensor.matmul(
                        ps, w_out[:, kb, mb * 128:(mb + 1) * 128],
                        z_t[:, kb, sl], start=(kb == 0), stop=(kb == 1))
                if last:
                    nc.scalar.copy(h4[:, mb, sl], ps)
                else:
                    nc.vector.tensor_tensor(out=res[:, mb, sl],
                                            in0=res[:, mb, sl], in1=ps,
                                            op=OP.add)

    if DBG:
        nc.sync.dma_start(io['dbg_h1'], res)
    # ---------------- final norm on cls column ----------------
    h44 = h4.rearrange("p k (b l) -> p k b l", b=BL)
    sq4 = smpool.tile([128, 2, BL], F32, tag="sq4")
    for kb in range(2):
        nc.vector.tensor_tensor(out=sq4[:, kb], in0=h44[:, kb, :, 0],
                                in1=h44[:, kb, :, 0], op=OP.mult)
    inv4_row = rms_inv_row(lambda kb: sq4[:, kb], BL, tag="row")
    inv4 = pmm.tile([128, BL], F32, tag="mm")
    nc.tensor.matmul(inv4, ones_row, inv4_row, start=True, stop=True)
    ocm = smpool.tile([128, 2, BL], F32, tag="ocm")
    for kb in range(2):
        nc.vector.scalar_tensor_tensor(
            out=ocm[:, kb], in0=h44[:, kb, :, 0],
            scalar=nfw_t[:, kb:kb + 1], in1=inv4, op0=OP.mult, op1=OP.mult)
    osb = smpool.tile([BL, D], F32, tag="osb", bufs=1)
    for kb in range(2):
        tps = paux.tile([BL, 128], F32, tag="mm16")
        nc.tensor.transpose(tps, ocm[:, kb], ident)
        nc.scalar.copy(osb[:, kb * 128:(kb + 1) * 128], tps)
    nc.sync.dma_start(o_out, osb)

    for c in reversed(ctxs):
        c.__exit__(None, None, None)


def build_kernel():
    nc = bass.Bass("TRN2", num_devices=NCORES)
    io = {
        'i_x': nc.dram_tensor("x", [BL, SEQ], F32, kind="ExternalInput").ap(),
        'i_pw': nc.dram_tensor("proj_w", [1, D], F32, kind="ExternalInput").ap(),
        'i_pb': nc.dram_tensor("proj_b", [D], F32, kind="ExternalInput").ap(),
        'i_cls': nc.dram_tensor("cls_token", [1, 1, D], F32, kind="ExternalInput").ap(),
        'i_pos': nc.dram_tensor("pos_embed", [1, L, D], F32, kind="ExternalInput").ap(),
        'i_nw': nc.dram_tensor("norm_w", [DEPTH, D], F32, kind="ExternalInput").ap(),
        'i_win': nc.dram_tensor("in_proj_w", [DEPTH, D, 2 * D], F32, kind="ExternalInput").ap(),
        'i_cw': nc.dram_tensor("conv_w", [DEPTH, D, K], F32, kind="ExternalInput").ap(),
        'i_cb': nc.dram_tensor("conv_b", [DEPTH, D], F32, kind="ExternalInput").ap(),
        'i_wxp': nc.dram_tensor("x_proj_w", [DEPTH, D, R + 2 * N], F32, kind="ExternalInput").ap(),
        'i_wdt': nc.dram_tensor("dt_proj_w", [DEPTH, R, D], F32, kind="ExternalInput").ap(),
        'i_dtb': nc.dram_tensor("dt_proj_b", [DEPTH, D], F32, kind="ExternalInput").ap(),
        'i_alog': nc.dram_tensor("A_log", [DEPTH, D, N], F32, kind="ExternalInput").ap(),
        'i_dsk': nc.dram_tensor("D_skip", [DEPTH, D], F32, kind="ExternalInput").ap(),
        'i_wout': nc.dram_tensor("out_proj_w", [DEPTH, D, D], F32, kind="ExternalInput").ap(),
        'i_nfw': nc.dram_tensor("norm_f_w", [D], F32, kind="ExternalInput").ap(),
        'o_out': nc.dram_tensor("out", [BL, D], F32, kind="ExternalOutput").ap(),
    }
    import os as _os
    if _os.environ.get('DEBUG_OUTS'):
        for nm in ('dbg_emb', 'dbg_hn', 'dbg_xc', 'dbg_dt', 'dbg_zs', 'dbg_g', 'dbg_h1'):
            io[nm] = nc.dram_tensor(nm, [128, 2, T], F32, kind="ExternalOutput").ap()
    with tile.TileContext(nc) as tc:
        _build(tc, nc, io)
    import os as _os
    if not _os.environ.get('NO_WSPLIT'):
        _split_waits(nc)
    return nc


_NC = None


def kernel(**inputs):
    global _NC
    if _NC is None:
        _NC = build_kernel()
    inp = {k: np.ascontiguousarray(np.asarray(v, dtype=np.float32))
           for k, v in inputs.items()}
    in_maps = []
    for core in range(NCORES):
        m = {k: v for k, v in inp.items() if k != 'x'}
        m['x'] = np.ascontiguousarray(inp['x'][core * BL:(core + 1) * BL])
        in_maps.append(m)
    r = run_bass_kernel_spmd(_NC, in_maps, core_ids=list(range(NCORES)))
    return np.concatenate([r.results[c]['out'] for c in range(NCORES)], axis=0)
